# revision 15
# baseline (speedup 1.0000x reference)
"""Trainium2 Bass kernel for nn_Attention_16612933501287.

Cross-attention block: c:(B=8,N=8,C=512,H=32,W=32), RMSNorm over C, fused
KV projection (512->1024), one query per (batch, head) attending over the
N=8 token axis at each spatial position, then output projection (512->512).

Sharding: data-parallel over B - one batch element per NeuronCore (8 cores).

v4 design (two-phase schedule, fp16 data path):
  host prep : c cast to fp16; fold g into Wkv; qv = emb[q]@Wq+bq; fold qv and
              the 1/sqrt(64) logit scale into per-batch Wd (512x8); k is never
              materialized; RMSNorm scale r = rsqrt(mean(c^2)+eps) precomputed
              per (n,p) and folded into the attention weights (the same
              fold-the-norm-into-weights trick the query path uses).
  schedule  : all 8 cp DMAs issued upfront (SP queue). PE emission staggers
              [draw_n logits] blocks between [vraw_{n-1} projection] blocks so
              cp-arrival gaps are filled. Each token's softmax chain
              (dots = draw*r -> e = exp -> num = e*r on DVE/ACT) runs right
              after its draw, bounces num through DRAM, and partition-
              broadcast DMAs (scalar+gpsimd queues) return numrep[128,4,P]
              fp16 well before that token's weighting slot. The weighting
              acc[:,ck] += vraw*numrep (DVE fp16 2x) follows each vraw block
              immediately; s_acc accumulates the softmax denominator, whose
              reciprocal (exp(-ln(s)), one pinned ACT table) broadcasts early.
  tail      : only norm (acc*srep) + output projection per h-half + bias/DMA.
Matmuls run fp16 moving/stationary (1 PE cycle/row); PSUM fp32.
"""

import numpy as np

import concourse.bass as bass
import concourse.bacc as bacc
import concourse.mybir as mybir
import concourse.tile as tile
from concourse.bass_utils import run_bass_kernel_spmd

# Pin the activation-function table: only natural_log_exp_and_others
# (ln/exp/square/identity/copy) is selectable, so the first-match chooser
# emits exactly one LoadActFuncSet instead of thrashing ln<->exp tables.
import concourse.hw_specs as _hw_specs
import concourse.bacc as _bacc_mod
_ORIG_GAT = _hw_specs.get_activation_tables


def _gat_pinned(arch):
    tabs = _ORIG_GAT(arch)
    return {name: (s if name == 'natural_log_exp_and_others' else set())
            for name, s in tabs.items()}


_bacc_mod.get_activation_tables = _gat_pinned

F32 = mybir.dt.float32
F16 = mybir.dt.float16
AF = mybir.ActivationFunctionType

B, N, C, H, W = 8, 8, 512, 32, 32
NH, HS = 8, 64
P = H * W           # 1024 spatial positions per core
NCC = C // 128      # 4 contraction chunks
EPS = 1e-6


def build_program():
    nc = bacc.Bacc()

    c_d = nc.declare_dram_parameter("c", [N, C, H, W], F16, isOutput=False)
    rr_d = nc.declare_dram_parameter("rr", [N, P], F16, isOutput=False)
    wv_d = nc.declare_dram_parameter("wv", [128, NCC, 512], F16, isOutput=False)
    wd_d = nc.declare_dram_parameter("wd", [128, NCC, NH], F16, isOutput=False)
    wo_d = nc.declare_dram_parameter("wout", [128, NCC, 512], F16, isOutput=False)
    bo_d = nc.declare_dram_parameter("bout", [128, NCC], F32, isOutput=False)
    out_d = nc.declare_dram_parameter("out", [C, H, W], F32, isOutput=True)
    nbounce_d = nc.dram_tensor("nbounce", [N, NH, P], F16)
    sbounce_d = nc.dram_tensor("sbounce", [NH, P], F16)

    cview = c_d[:].rearrange("n (cc k) h w -> n k cc (h w)", k=128)
    oview = out_d[:].rearrange("(do k) h w -> do k (h w)", k=128)

    with tile.TileContext(nc) as tc:
        with (
            tc.tile_pool(name="consts", bufs=1) as consts,
            tc.tile_pool(name="store", bufs=1) as store,
            tc.tile_pool(name="cp_pool", bufs=8) as cp_pool,
            tc.tile_pool(name="vraw_pool", bufs=2) as vraw_pool,
            tc.tile_pool(name="nrep_pool", bufs=3) as nrep_pool,
            tc.tile_pool(name="ch_pool", bufs=3) as ch_pool,
            tc.tile_pool(name="vw_pool", bufs=4) as vw_pool,
            tc.tile_pool(name="ps_stat", bufs=2, space="PSUM") as ps_stat,
            tc.tile_pool(name="ps_big", bufs=4, space="PSUM") as ps_big,
        ):
            # === BEGIN BODY ===
            wd_sb = consts.tile([128, NCC, NH], F16)
            nc.sync.dma_start(out=wd_sb, in_=wd_d[:])
            wv_sb = consts.tile([128, NCC, 512], F16)
            wo_sb = consts.tile([128, NCC, 512], F16)
            bo_sb = consts.tile([128, NCC], F32)

            acc = store.tile([128, NCC, P], F16)
            s_acc = store.tile([NH, P], F16)
            srep = store.tile([128, NCC, P], F16)

            cps, rs = [], []
            for n in range(N):
                cp = cp_pool.tile([128, NCC, P], F16, name=f"cp_{n}",
                                  tag="cp")
                nc.sync.dma_start(out=cp, in_=cview[n])
                cps.append(cp)
                r_sb = ch_pool.tile([NH, P], F16, name=f"r_{n}", tag="r")
                nc.gpsimd.dma_start(out=r_sb,
                                    in_=rr_d[n].partition_broadcast(NH))
                rs.append(r_sb)
                if n == 0:
                    # big consts queued right behind cp_0 on the other HWDGE
                    nc.scalar.dma_start(out=wv_sb, in_=wv_d[:])
                if n == 2:
                    nc.scalar.dma_start(out=wo_sb, in_=wo_d[:])
                    nc.scalar.dma_start(out=bo_sb, in_=bo_d[:])

            def emit_stats(n):
                """Logits + softmax chain + bounce for token n."""
                cp, r_sb = cps[n], rs[n]
                stats = ps_stat.tile([NH, P], F32, name=f"stats_{n}",
                                     tag="stats")
                for cc in range(NCC):
                    for h in range(2):
                        nc.tensor.matmul(
                            stats[:, h * 512:(h + 1) * 512],
                            wd_sb[:, cc, :],
                            cp[:, cc, h * 512:(h + 1) * 512],
                            start=(cc == 0),
                            stop=(cc == NCC - 1),
                        )
                dots = ch_pool.tile([NH, P], F16, name=f"dots_{n}", tag="dots")
                nc.vector.tensor_mul(out=dots, in0=stats, in1=r_sb)
                e_sb = ch_pool.tile([NH, P], F16, name=f"e_{n}", tag="e")
                nc.scalar.activation(out=e_sb, in_=dots, func=AF.Exp)
                num = ch_pool.tile([NH, P], F16, name=f"num_{n}", tag="num")
                nc.vector.tensor_mul(out=num, in0=e_sb, in1=r_sb)
                if n == 0:
                    nc.vector.tensor_scalar_add(out=s_acc, in0=e_sb,
                                                scalar1=0.0)
                else:
                    nc.vector.tensor_add(out=s_acc, in0=s_acc, in1=e_sb)
                if n == N - 1:
                    # denominator reciprocal + broadcast, all well before the
                    # tail; token 7's weights are pre-normalized by srecip so
                    # the acc normalization need not wait for them
                    lns = store.tile([NH, P], F16)
                    nc.scalar.activation(out=lns, in_=s_acc, func=AF.Ln)
                    srecip = store.tile([NH, P], F16)
                    nc.scalar.activation(out=srecip, in_=lns, func=AF.Exp,
                                         scale=-1.0)
                    nc.gpsimd.dma_start(out=sbounce_d[:], in_=srecip)
                    for j in range(2):
                        nc.sync.dma_start(
                            out=srep[j * 64:(j + 1) * 64],
                            in_=sbounce_d[j::2, :].partition_broadcast(64),
                        )
                nc.gpsimd.dma_start(out=nbounce_d[n], in_=num)

            def emit_vraw(n):
                """V projection + weighting for token n."""
                cp = cps[n]
                nrep = nrep_pool.tile([128, NCC, P], F16, name=f"nrep_{n}",
                                      tag="nrep")
                vraw = vraw_pool.tile([128, NCC, P], F16, name=f"vraw_{n}",
                                      tag="vraw")
                for ck in range(NCC):
                    for h in range(2):
                        v_ps = ps_big.tile([128, 512], F32, tag="v_ps",
                                           name=f"v_ps_{n}_{ck}_{h}")
                        for cc in range(NCC):
                            nc.tensor.matmul(
                                v_ps,
                                wv_sb[:, cc, ck * 128:(ck + 1) * 128],
                                cp[:, cc, h * 512:(h + 1) * 512],
                                start=(cc == 0),
                                stop=(cc == NCC - 1),
                            )
                        nc.scalar.copy(
                            out=vraw[:, ck, h * 512:(h + 1) * 512], in_=v_ps)
                # broadcast numrep back (Pool queue: ordered after the
                # bounce write without blocking the HWDGE sequencers)
                for j in range(2):
                    nc.gpsimd.dma_start(
                        out=nrep[j * 64:(j + 1) * 64],
                        in_=nbounce_d[n, j::2, :].partition_broadcast(64))
                # weighting; token 6 is followed by the acc
                # normalization (token 7's weights are pre-normalized)
                hsplits = [slice(0, P)] if n < N - 1 else \
                    [slice(0, 512), slice(512, P)]
                for hs_ in hsplits:
                    for ck in range(NCC):
                        # Pool absorbs part of the weighting so DVE is not
                        # oversubscribed at the tail
                        eng = nc.vector
                        if n == 0:
                            eng.tensor_mul(out=acc[:, ck, hs_],
                                           in0=vraw[:, ck, hs_],
                                           in1=nrep[:, ck, hs_])
                        else:
                            vw = vw_pool.tile([128, P], F16,
                                              name=f"vw_{n}_{ck}_{hs_.start}",
                                              tag="vw")
                            vwv = vw[:, :hs_.stop - hs_.start]
                            eng.tensor_mul(out=vwv,
                                           in0=vraw[:, ck, hs_],
                                           in1=nrep[:, ck, hs_])
                            eng.tensor_add(out=acc[:, ck, hs_],
                                           in0=acc[:, ck, hs_], in1=vwv)


            # staggered emission: draws fill the cp-DMA arrival gaps
            emit_stats(0)
            emit_stats(1)
            for n in range(N):
                if n + 2 < N:
                    emit_vraw(n)
                    emit_stats(n + 2)
                else:
                    emit_vraw(n)

            # ======================== tail ========================
            for h in range(2):
                hs_ = slice(h * 512, (h + 1) * 512)
                for ck in range(NCC):
                    nc.vector.tensor_mul(out=acc[:, ck, hs_],
                                         in0=acc[:, ck, hs_],
                                         in1=srep[:, ck, hs_])
                for do in range(NCC):
                    ot_ps = ps_big.tile([128, 512], F32, tag="v_ps",
                                        name=f"ot_ps_{do}_{h}")
                    for di in range(NCC):
                        nc.tensor.matmul(
                            ot_ps,
                            wo_sb[:, di, do * 128:(do + 1) * 128],
                            acc[:, di, hs_],
                            start=(di == 0),
                            stop=(di == NCC - 1),
                        )
                    ot_sb = vw_pool.tile([128, 512], F32,
                                         name=f"ot_sb_{do}_{h}", tag="ot")
                    nc.scalar.activation(
                        out=ot_sb, in_=ot_ps,
                        func=AF.Identity, bias=bo_sb[:, do:do + 1],
                    )
                    nc.sync.dma_start(out=oview[do, :, hs_], in_=ot_sb)
            # === END BODY ===

    nc.finalize()
    return nc


_CACHE = {}


def _get_nc():
    if "nc" not in _CACHE:
        _CACHE["nc"] = build_program()
    return _CACHE["nc"]


def _prep_inputs(q, c, emb, Wq, bq, Wkv, Wout, bout, g):
    q = np.asarray(q)
    c = np.asarray(c, dtype=np.float32)
    emb = np.asarray(emb, dtype=np.float32)
    Wq = np.asarray(Wq, dtype=np.float32)
    bq = np.asarray(bq, dtype=np.float32)
    Wkv = np.asarray(Wkv, dtype=np.float32)
    Wout = np.asarray(Wout, dtype=np.float32)
    bout = np.asarray(bout, dtype=np.float32)
    g = np.asarray(g, dtype=np.float32)

    qv = emb[q] @ Wq + bq                                   # (B, 512)
    qvs = qv.reshape(B, NH, HS).astype(np.float32) * np.float32(HS ** -0.5)
    Wkv_g = (g[:, None] * Wkv).astype(np.float32)
    Wk3 = Wkv_g[:, :C].reshape(C, NH, HS)
    Wv = np.ascontiguousarray(Wkv_g[:, C:])                 # (512, 512)
    Wd = np.einsum('chs,bhs->bch', Wk3, qvs).astype(np.float32)  # (B, 512, 8)

    wv_host = np.ascontiguousarray(
        Wv.reshape(NCC, 128, 512).transpose(1, 0, 2)).astype(np.float16)
    wd_host = np.ascontiguousarray(
        Wd.reshape(B, NCC, 128, NH).transpose(0, 2, 1, 3)).astype(np.float16)
    wout_host = np.ascontiguousarray(
        Wout.reshape(NCC, 128, 512).transpose(1, 0, 2)).astype(np.float16)
    bout_host = np.ascontiguousarray(bout.reshape(NCC, 128).T)  # [k, do]

    c16 = c.astype(np.float16)
    # RMSNorm scale folded into the attention weights: r[b,n,p]
    ms = np.mean(np.square(c), axis=2)                      # (B, N, H, W)
    rr = (1.0 / np.sqrt(ms + EPS)).reshape(B, N, P).astype(np.float16)

    in_maps = []
    for b in range(B):
        in_maps.append({
            "c": np.ascontiguousarray(c16[b]),
            "rr": np.ascontiguousarray(rr[b]),
            "wv": wv_host,
            "wd": np.ascontiguousarray(wd_host[b]),
            "wout": wout_host,
            "bout": bout_host,
        })
    return in_maps


def kernel(**inputs) -> np.ndarray:
    nc = _get_nc()
    in_maps = _prep_inputs(**inputs)
    res = run_bass_kernel_spmd(nc, in_maps, list(range(B)))
    return np.stack([res.results[b]["out"] for b in range(B)], axis=0)


if __name__ == "__main__":
    nc = build_program()
    print("program built ok")
